# revision 15
# baseline (speedup 1.0000x reference)
"""Trainium2 Bass kernel for nn_CVRPGNNBase (3-layer GAT + edge/node heads + graph pooling).

Self-contained: builds and runs an 8-core SPMD Bass kernel via
concourse (bass/tile) on the axon-attached TRN2 chip.

Sharding:
  - nodes are split into 8 contiguous ranges of NSH (last range padded);
    GAT edges are sharded by destination node and sorted by dst on the host.
  - each core computes h = xn @ W_aug for its own nodes (W_aug carries fused
    columns for the attention dot products a_s/a_d), then an AllGather
    replicates the per-node rows; per-edge rows are fetched with dma_gather.
  - segment softmax/weighted-sum run as one-hot matmuls accumulating in PSUM
    (128 dst nodes per tile; exp weights applied to the gathered rows).
  - the edge classifier is sharded by edge (bf16 transposed gathers);
    graph pooling is resharded by graph id (G/8 graphs per core).
"""

import sys

sys.path.insert(0, "/opt/trn_rl_repo")

import numpy as np

import concourse.bacc as bacc
import concourse.tile as tile
from concourse import mybir

F32 = mybir.dt.float32
BF16 = mybir.dt.bfloat16
I16 = mybir.dt.int16
I32 = mybir.dt.int32
AF = mybir.ActivationFunctionType
OP = mybir.AluOpType

NCORES = 8
EPS = 1e-5


# ---------------------------------------------------------------- host utils

def _idx16(flat, n):
    """dma_gather index layout: flat[k] -> [k % 16, k // 16], tiled to 128 rows."""
    flat = np.asarray(flat, np.int64)
    assert flat.size == n and n % 16 == 0
    a = flat.astype(np.int16).reshape(-1, 16).T
    return np.tile(a, (8, 1))


def _edge_layout(flat):
    """per-edge values in gather-output layout [p, b]: edge k -> [k%128, k//128]."""
    flat = np.asarray(flat)
    assert flat.size % 128 == 0
    return np.ascontiguousarray(flat.reshape(-1, 128).T)


def _ceil(a, b):
    return -(-a // b)


class Dims:
    def __init__(self, N, E, G, NF, EF, HID, H, T_FIX, SLOTG):
        self.N, self.E, self.G = N, E, G
        self.NF, self.EF, self.HID, self.H = NF, EF, HID, H
        self.NSH = _ceil(N, NCORES * 128) * 128
        self.NT = self.NSH // 128
        self.NTOT = self.NSH * NCORES
        self.ESH = E // NCORES
        self.ESHP = _ceil(self.ESH, 2048) * 2048
        self.T_FIX = T_FIX
        self.B_FIX = T_FIX // 128
        nch = _ceil(self.B_FIX, 8)
        base, rem = divmod(self.B_FIX, nch)
        self.CHUNKS = [base + (1 if i < rem else 0) for i in range(nch)]
        self.SLOTG = SLOTG
        self.GSH = G // NCORES
        self.NSLOT = self.GSH * self.SLOTG
        self.NBLK = self.NSLOT // 128
        F0, FC = HID, HID * H
        self.layers = [
            dict(fin=F0, nh=H, hc=HID, F=FC, elu=True),
            dict(fin=FC, nh=H, hc=HID, F=FC, elu=True),
            dict(fin=FC, nh=1, hc=HID, F=HID, elu=False),
        ]
        for L in self.layers:
            L["nout"] = L["F"] + 2 * L["nh"]
            L["row"] = _ceil(L["nout"], 64) * 64
            L["kc"] = _ceil(L["fin"], 128)
            L["nrhs"] = L["F"] + L["nh"]
        self.ZROW = self.NTOT      # zero row in h_aug / xn3 arrays
        self.ZAD = self.NSH        # zero row in ad_pad
        self.XNEG = self.NTOT + 1  # -BIG row in xn3f

    def sig(self):
        return (self.N, self.E, self.G, self.NF, self.EF, self.HID, self.H,
                self.T_FIX, self.SLOTG)


# ---------------------------------------------------------------- device build

def build_program(d: Dims):
    nc = bacc.Bacc("TRN2", target_bir_lowering=False, debug=False,
                   num_devices=NCORES)
    NT, B_FIX, T_FIX = d.NT, d.B_FIX, d.T_FIX
    HID, GSH = d.HID, d.GSH

    def din(name, shape, dt=F32):
        return nc.dram_tensor(name, shape, dt, kind="ExternalInput")

    def dout(name, shape, dt=F32):
        return nc.dram_tensor(name, shape, dt, kind="ExternalOutput")

    # ---- inputs
    xT = din("xT", [d.NF, d.NSH])
    eaT = din("eaT", [d.EF, d.ESHP])
    src16 = din("src16", [128, NT * T_FIX // 16], I16)
    dst16 = din("dst16", [128, NT * T_FIX // 16], I16)
    dstl = din("dstl", [128, NT * B_FIX])
    cls_s16 = din("cls_s16", [128, d.ESHP // 16], I16)
    cls_d16 = din("cls_d16", [128, d.ESHP // 16], I16)
    slotz16 = din("slotz16", [128, d.NSLOT // 16], I16)
    slotn16 = din("slotn16", [128, d.NSLOT // 16], I16)
    gsel_in = din("gsel", [128, d.NBLK, GSH])
    invcnt = din("invcnt", [GSH, 1])
    ident = din("ident", [128, 128])
    ne_w = din("ne_w", [d.NF, HID])
    ne_b_bc = din("ne_b_bc", [128, HID])
    ne_g_bc = din("ne_g_bc", [128, HID])
    ne_be_bc = din("ne_be_bc", [128, HID])
    ee_w = din("ee_w", [d.EF, 32])
    ee_b4 = din("ee_b4", [128, 1])
    bd_sum = din("bd_sum", [128, 4])
    bd_bc = din("bd_bc", [4, 128])
    Wl = [din(f"W{i}", [128, L["kc"], L["nout"]]) for i, L in enumerate(d.layers)]
    bl = [din(f"b{i}_bc", [128, L["F"]]) for i, L in enumerate(d.layers)]
    ec1a = din("ec1a", [64, 64], BF16)
    ec1b = din("ec1b", [64, 64], BF16)
    ec1c = din("ec1c", [32, 64], BF16)
    ec1_bp = din("ec1_bp", [64, 1])
    ec2 = din("ec2", [64, 64], BF16)
    ec2_bp = din("ec2_bp", [64, 1])
    ec3 = din("ec3", [64, 1], BF16)
    ec3_bp = din("ec3_bp", [1, 1])
    ns1 = din("ns1", [64, 64])
    ns1_bp = din("ns1_bp", [64, 1])
    ns2 = din("ns2", [64, 1])
    ns2_bp = din("ns2_bp", [1, 1])

    # ---- outputs
    xn_out = dout("xn_out", [d.NSH, HID])
    np_out = dout("np_out", [1, d.NSH])
    ep_out = dout("ep_out", [1, d.ESHP])
    gf_out = dout("gf_out", [GSH, 2 * HID])

    # ---- internal dram
    ag_in = [nc.dram_tensor(f"ag_in{i}", [d.NSH, L["row"]], F32)
             for i, L in enumerate(d.layers)]
    h_aug = [nc.dram_tensor(f"h_aug{i}", [d.NTOT + 1, L["row"]], F32,
                            addr_space="Shared")
             for i, L in enumerate(d.layers)]
    ad_pad = [nc.dram_tensor(f"ad_pad{i}", [d.NSH + 1, 64], F32)
              for i in range(3)]
    n_ef_ch = d.ESHP // 2048
    ef_store = nc.dram_tensor("ef_store", [n_ef_ch, 4, 32, 512], BF16)
    ag3_in = nc.dram_tensor("ag3_in", [d.NSH, HID], F32)
    xn3f = nc.dram_tensor("xn3f", [d.NTOT + 2, HID], F32, addr_space="Shared")
    ag4_in = nc.dram_tensor("ag4_in", [d.NSH, 128], BF16)
    xn3bf = nc.dram_tensor("xn3bf", [d.NTOT + 1, 128], BF16,
                           addr_space="Shared")

    groups = [list(range(NCORES))]

    with tile.TileContext(nc) as tc:
        with (
            tc.tile_pool(name="persist", bufs=1) as pp,
            tc.tile_pool(name="work", bufs=2) as wp,
            tc.tile_pool(name="gat", bufs=2) as gp,
            tc.tile_pool(name="ps_a", bufs=2, space="PSUM") as ps_a,
            tc.tile_pool(name="ps_b", bufs=2, space="PSUM") as ps_b,
            tc.tile_pool(name="ps_c", bufs=2, space="PSUM") as ps_c,
        ):
            def pload(src, shape, dt=F32):
                t = pp.tile(shape, dt, name=f"p_{src.name}", tag=f"p_{src.name}")
                nc.sync.dma_start(t[:], src[:])
                return t

            # ---------- persistent loads
            t_ident = pload(ident, [128, 128])
            t_xT = pload(xT, [d.NF, d.NSH])
            t_src16 = pload(src16, [128, NT * T_FIX // 16], I16)
            t_dst16 = pload(dst16, [128, NT * T_FIX // 16], I16)
            t_dstl = pload(dstl, [128, NT * B_FIX])
            t_slotz16 = pload(slotz16, [128, d.NSLOT // 16], I16)
            t_slotn16 = pload(slotn16, [128, d.NSLOT // 16], I16)
            t_ne_w = pload(ne_w, [d.NF, HID])
            t_ne_b = pload(ne_b_bc, [128, HID])
            t_ne_g = pload(ne_g_bc, [128, HID])
            t_ne_be = pload(ne_be_bc, [128, HID])
            t_ee_w = pload(ee_w, [d.EF, 32])
            t_ee_b4 = pload(ee_b4, [128, 1])
            t_bd_sum = pload(bd_sum, [128, 4])
            t_bd_bc = pload(bd_bc, [4, 128])
            t_W = [pload(Wl[i], [128, L["kc"], L["nout"]])
                   for i, L in enumerate(d.layers)]
            t_b = [pload(bl[i], [128, L["F"]]) for i, L in enumerate(d.layers)]
            t_ec1a = pload(ec1a, [64, 64], BF16)
            t_ec1b = pload(ec1b, [64, 64], BF16)
            t_ec1c = pload(ec1c, [32, 64], BF16)
            t_ec1bp = pload(ec1_bp, [64, 1])
            t_ec2 = pload(ec2, [64, 64], BF16)
            t_ec2bp = pload(ec2_bp, [64, 1])
            t_ec3 = pload(ec3, [64, 1], BF16)
            t_ec3bp = pload(ec3_bp, [1, 1])
            t_ns1 = pload(ns1, [64, 64])
            t_ns1b = pload(ns1_bp, [64, 1])
            t_ns2 = pload(ns2, [64, 1])
            t_ns2b = pload(ns2_bp, [1, 1])
            t_gsel = pload(gsel_in, [128, d.NBLK, GSH])
            t_invc = pload(invcnt, [GSH, 1])

            # iota row 0..127, replicated on every partition
            t_iota_i = pp.tile([128, 128], I32)
            nc.gpsimd.iota(t_iota_i[:], pattern=[[1, 128]], base=0,
                           channel_multiplier=0)
            t_iota = pp.tile([128, 128], F32)
            nc.vector.tensor_copy(t_iota[:], t_iota_i[:])

            t_zrow = pp.tile([1, 512], F32)
            nc.vector.memset(t_zrow[:], 0.0)
            t_eps = pp.tile([128, 1], F32)
            nc.vector.memset(t_eps[:], EPS)

            # per-layer transposed inputs: [128, kc, NSH]
            t_xnT = [pp.tile([128, L["kc"], d.NSH], F32, name=f"xnT{i}", tag=f"xnT{i}")
                     for i, L in enumerate(d.layers)]
            t_xn3T = pp.tile([128, 1, d.NSH], F32)

            # ---------- node encoder (own shard, node-major free-dim LN)
            for t in range(NT):
                ts = slice(t * 128, (t + 1) * 128)
                ps = ps_a.tile([128, HID], F32, space="PSUM", tag="mm")
                nc.tensor.matmul(ps[:], lhsT=t_xT[:, ts], rhs=t_ne_w[:],
                                 start=True, stop=True)
                y = wp.tile([128, HID], F32, tag="ency")
                nc.vector.tensor_tensor(y[:], ps[:], t_ne_b[:], op=OP.add)
                nc.vector.tensor_scalar_max(y[:], y[:], 0.0)
                mu = wp.tile([128, 1], F32, tag="encmu")
                nc.vector.tensor_reduce(mu[:], y[:], axis=mybir.AxisListType.X,
                                        op=OP.add)
                nc.vector.tensor_scalar_mul(mu[:], mu[:], 1.0 / HID)
                z = wp.tile([128, HID], F32, tag="encz")
                nc.vector.tensor_scalar(z[:], y[:], mu[:], None, op0=OP.subtract)
                sq = wp.tile([128, HID], F32, tag="encsq")
                nc.scalar.square(sq[:], z[:])
                va = wp.tile([128, 1], F32, tag="encva")
                nc.vector.tensor_reduce(va[:], sq[:], axis=mybir.AxisListType.X,
                                        op=OP.add)
                nc.vector.tensor_scalar_mul(va[:], va[:], 1.0 / HID)
                sd = wp.tile([128, 1], F32, tag="encsd")
                nc.scalar.activation(sd[:], va[:], AF.Sqrt, bias=t_eps[:])
                nc.vector.reciprocal(sd[:], sd[:])
                nc.vector.tensor_scalar_mul(z[:], z[:], sd[:])
                nc.vector.tensor_tensor(z[:], z[:], t_ne_g[:], op=OP.mult)
                nc.vector.tensor_tensor(z[:], z[:], t_ne_be[:], op=OP.add)
                pt = ps_c.tile([128, 128], F32, space="PSUM", tag="tr")
                nc.tensor.transpose(pt[0:HID, :], z[:, 0:HID], t_ident[:])
                nc.scalar.activation(t_xnT[0][0:HID, 0, ts], pt[0:HID, :],
                                     AF.Copy)

            # ---------- edge feature encoder (4-stacked feat-major LN)
            for ci in range(n_ef_ch):
                ea_ch = wp.tile([d.EF, 2048], F32, tag="ea_ch", bufs=1)
                nc.sync.dma_start(ea_ch[:],
                                  eaT[:, ci * 2048:(ci + 1) * 2048])
                raw4 = wp.tile([128, 512], F32, tag="efraw")
                for j in range(4):
                    pse = ps_a.tile([32, 512], F32, space="PSUM", tag="mm")
                    nc.tensor.matmul(
                        pse[:], lhsT=t_ee_w[:],
                        rhs=ea_ch[:, j * 512:(j + 1) * 512],
                        start=True, stop=True)
                    nc.scalar.activation(raw4[32 * j:32 * (j + 1), :], pse[:],
                                         AF.Relu,
                                         bias=t_ee_b4[32 * j:32 * (j + 1), :])
                sq4 = wp.tile([128, 512], F32, tag="efsq", bufs=1)
                nc.scalar.square(sq4[:], raw4[:])
                ps_s = ps_a.tile([4, 512], F32, space="PSUM", tag="mm")
                nc.tensor.matmul(ps_s[:], lhsT=t_bd_sum[:], rhs=raw4[:],
                                 start=True, stop=True)
                ps_q = ps_a.tile([4, 512], F32, space="PSUM", tag="mm")
                nc.tensor.matmul(ps_q[:], lhsT=t_bd_sum[:], rhs=sq4[:],
                                 start=True, stop=True)
                mean = wp.tile([4, 512], F32, tag="efmean", bufs=1)
                nc.scalar.activation(mean[:], ps_s[:], AF.Copy, scale=1.0 / 32)
                ivar = wp.tile([4, 512], F32, tag="efivar", bufs=1)
                nc.scalar.activation(ivar[:], ps_q[:], AF.Copy, scale=1.0 / 32)
                m2 = wp.tile([4, 512], F32, tag="efm2", bufs=1)
                nc.vector.tensor_tensor(m2[:], mean[:], mean[:], op=OP.mult)
                nc.vector.tensor_tensor(ivar[:], ivar[:], m2[:], op=OP.subtract)
                nc.scalar.activation(ivar[:], ivar[:], AF.Sqrt, bias=t_eps[0:4, :])
                nc.vector.reciprocal(ivar[:], ivar[:])
                ps_mb = ps_a.tile([128, 512], F32, space="PSUM", tag="mm")
                nc.tensor.matmul(ps_mb[:], lhsT=t_bd_bc[:], rhs=mean[:],
                                 start=True, stop=True)
                ps_ib = ps_a.tile([128, 512], F32, space="PSUM", tag="mm")
                nc.tensor.matmul(ps_ib[:], lhsT=t_bd_bc[:], rhs=ivar[:],
                                 start=True, stop=True)
                zn = wp.tile([128, 512], F32, tag="efzn", bufs=1)
                nc.vector.tensor_tensor(zn[:], raw4[:], ps_mb[:], op=OP.subtract)
                nc.vector.tensor_tensor(zn[:], zn[:], ps_ib[:], op=OP.mult)
                efb = wp.tile([128, 512], BF16, tag="efbf")
                nc.vector.tensor_copy(efb[:], zn[:])
                nc.sync.dma_start(
                    ef_store[ci].rearrange("g f e -> (g f) e")[:], efb[:])

            # ---------- GAT layers
            for li, L in enumerate(d.layers):
                row, nout, F, nh, nrhs = (L["row"], L["nout"], L["F"], L["nh"],
                                          L["nrhs"])
                hc, kc = L["hc"], L["kc"]
                xin = t_xnT[li]
                for t in range(NT):
                    ts = slice(t * 128, (t + 1) * 128)
                    ph = ps_a.tile([128, nout], F32, space="PSUM", tag="mm")
                    for k in range(kc):
                        kw = min(128, L["fin"] - k * 128)
                        nc.tensor.matmul(ph[:], lhsT=xin[0:kw, k, ts],
                                         rhs=t_W[li][0:kw, k, 0:nout],
                                         start=(k == 0), stop=(k == kc - 1))
                    hst = wp.tile([128, row], F32, tag="hst")
                    nc.scalar.activation(hst[:, 0:nout], ph[:], AF.Copy)
                    if row > nout:
                        nc.vector.memset(hst[:, nout:row], 0.0)
                    nc.sync.dma_start(ag_in[li][ts, :], hst[:])
                    adst = wp.tile([128, 64], F32, tag="adst")
                    nc.vector.memset(adst[:], 0.0)
                    nc.vector.tensor_copy(adst[:, 0:nh],
                                          hst[:, F + nh:F + 2 * nh])
                    nc.sync.dma_start(ad_pad[li][ts, :], adst[:])
                nc.sync.dma_start(h_aug[li][d.ZROW:d.ZROW + 1, :],
                                  t_zrow[:, 0:row])
                nc.sync.dma_start(ad_pad[li][d.ZAD:d.ZAD + 1, :],
                                  t_zrow[:, 0:64])
                nc.gpsimd.collective_compute(
                    "AllGather", OP.bypass, replica_groups=groups,
                    ins=[ag_in[li][:]], outs=[h_aug[li][0:d.NTOT, :]])

                for t in range(NT):
                    acc = ps_b.tile([128, nrhs], F32, space="PSUM", tag="gacc")
                    c0 = 0
                    for nb in d.CHUNKS:
                        e0 = t * T_FIX + c0 * 128
                        ne = nb * 128
                        hg = gp.tile([128, nb, row], F32, tag="hg")
                        nc.gpsimd.dma_gather(
                            hg[:], h_aug[li][:],
                            t_src16[:, e0 // 16:(e0 + ne) // 16], ne, ne, row,
                            single_packet=False)
                        ad = gp.tile([128, nb, 64], F32, tag="ad")
                        nc.gpsimd.dma_gather(
                            ad[:], ad_pad[li][:],
                            t_dst16[:, e0 // 16:(e0 + ne) // 16], ne, ne, 64,
                            single_packet=False)
                        epre = gp.tile([128, nb, nh], F32, tag="epre")
                        nc.vector.tensor_tensor(epre[:], hg[:, :, F:F + nh],
                                                ad[:, :, 0:nh], op=OP.add)
                        esc = gp.tile([128, nb, nh], F32, tag="esc")
                        nc.vector.tensor_scalar_mul(esc[:], epre[:], 0.2)
                        nc.vector.tensor_tensor(esc[:], esc[:], epre[:],
                                                op=OP.max)
                        ex = gp.tile([128, nb, nh], F32, tag="ex")
                        nc.scalar.activation(ex[:], esc[:], AF.Exp)
                        rhs = gp.tile([128, nb, nrhs], F32, tag="rhs")
                        if nh > 1:
                            nc.vector.tensor_tensor(
                                rhs[:, :, 0:F].rearrange(
                                    "p b (h c) -> p b h c", h=nh),
                                hg[:, :, 0:F].rearrange(
                                    "p b (h c) -> p b h c", h=nh),
                                ex[:, :, :, None].to_broadcast(
                                    [128, nb, nh, hc]),
                                op=OP.mult)
                        else:
                            nc.vector.tensor_tensor(
                                rhs[:, :, 0:F], hg[:, :, 0:F],
                                ex[:, :, 0:1].to_broadcast([128, nb, F]),
                                op=OP.mult)
                        nc.vector.tensor_copy(rhs[:, :, F:F + nh], ex[:])
                        p0 = gp.tile([128, nb, 128], F32, tag="p0")
                        nc.vector.tensor_tensor(
                            p0[:],
                            t_dstl[:, t * B_FIX + c0:t * B_FIX + c0 + nb,
                                   None].to_broadcast([128, nb, 128]),
                            t_iota[:, None, :].to_broadcast([128, nb, 128]),
                            op=OP.is_equal)
                        for b in range(nb):
                            nc.tensor.matmul(
                                acc[:], lhsT=p0[:, b, :], rhs=rhs[:, b, :],
                                start=(c0 + b == 0),
                                stop=(c0 + b == B_FIX - 1))
                        c0 += nb
                    accs = wp.tile([128, nrhs], F32, tag="gaccs")
                    nc.scalar.activation(accs[:], acc[:], AF.Copy)
                    rec = wp.tile([128, nh], F32, tag="grec")
                    nc.vector.reciprocal(rec[:], accs[:, F:F + nh])
                    outt = wp.tile([128, F], F32, tag="gout")
                    for h in range(nh):
                        nc.vector.tensor_scalar_mul(
                            outt[:, h * hc:(h + 1) * hc],
                            accs[:, h * hc:(h + 1) * hc], rec[:, h:h + 1])
                    nc.vector.tensor_tensor(outt[:], outt[:], t_b[li][:],
                                            op=OP.add)
                    if L["elu"]:
                        tm = wp.tile([128, F], F32, tag="gelu")
                        nc.vector.tensor_scalar_min(tm[:], outt[:], 0.0)
                        nc.scalar.activation(tm[:], tm[:], AF.Exp)
                        nc.vector.tensor_scalar_max(outt[:], outt[:], 0.0)
                        nc.vector.tensor_tensor(outt[:], outt[:], tm[:],
                                                op=OP.add)
                        nc.vector.tensor_scalar(outt[:], outt[:], 1.0, None,
                                                op0=OP.subtract)
                    ts = slice(t * 128, (t + 1) * 128)
                    xnext = t_xnT[li + 1] if li < 2 else t_xn3T
                    for fc in range(_ceil(F, 128)):
                        fw = min(128, F - fc * 128)
                        ptr = ps_c.tile([128, 128], F32, space="PSUM", tag="tr")
                        nc.tensor.transpose(ptr[0:fw, :],
                                            outt[:, fc * 128:fc * 128 + fw],
                                            t_ident[:])
                        nc.scalar.activation(xnext[0:fw, fc, ts], ptr[0:fw, :],
                                             AF.Copy)
                    if li == 2:
                        nc.sync.dma_start(xn_out[ts, :], outt[:])
                        nc.sync.dma_start(ag3_in[ts, :], outt[:])
                        xb = wp.tile([128, 128], BF16, tag="xnbf")
                        nc.vector.tensor_copy(xb[:, 0:HID], outt[:])
                        nc.vector.memset(xb[:, HID:128], 0.0)
                        nc.sync.dma_start(ag4_in[ts, :], xb[:])

            # ---------- final allgathers (+ special rows)
            nc.sync.dma_start(xn3f[d.ZROW:d.ZROW + 1, :], t_zrow[:, 0:HID])
            t_neg = pp.tile([1, HID], F32)
            nc.vector.memset(t_neg[:], -1e30)
            nc.sync.dma_start(xn3f[d.XNEG:d.XNEG + 1, :], t_neg[:])
            t_zbf = pp.tile([1, 128], BF16)
            nc.vector.memset(t_zbf[:], 0.0)
            nc.sync.dma_start(xn3bf[d.ZROW:d.ZROW + 1, :], t_zbf[:])
            nc.gpsimd.collective_compute(
                "AllGather", OP.bypass, replica_groups=groups,
                ins=[ag3_in[:]], outs=[xn3f[0:d.NTOT, :]])
            nc.gpsimd.collective_compute(
                "AllGather", OP.bypass, replica_groups=groups,
                ins=[ag4_in[:]], outs=[xn3bf[0:d.NTOT, :]])

            # ---------- node scorer (own shard, transposed layout)
            o = 0
            while o < d.NSH:
                w = min(512, d.NSH - o)
                psn = ps_a.tile([64, 512], F32, space="PSUM", tag="mm")
                nc.tensor.matmul(psn[:, 0:w], lhsT=t_ns1[:],
                                 rhs=t_xn3T[0:64, 0, o:o + w],
                                 start=True, stop=True)
                n1 = wp.tile([64, 512], F32, tag="nsh")
                nc.scalar.activation(n1[:, 0:w], psn[:, 0:w], AF.Relu,
                                     bias=t_ns1b[:])
                psn2 = ps_a.tile([1, 512], F32, space="PSUM", tag="mm")
                nc.tensor.matmul(psn2[:, 0:w], lhsT=t_ns2[:], rhs=n1[:, 0:w],
                                 start=True, stop=True)
                npr = wp.tile([1, 512], F32, tag="rowout")
                nc.vector.tensor_scalar(npr[:, 0:w], psn2[:, 0:w], t_ns2b[:],
                                        None, op0=OP.add)
                nc.sync.dma_start(np_out[:, o:o + w], npr[:, 0:w])
                o += w

            # ---------- edge classifier (bf16 transposed gathers)
            for ci in range(d.ESHP // 2048):
                ci_s = gp.tile([128, 128], I16, tag="ci_s")
                nc.sync.dma_start(ci_s[:], cls_s16[:, ci * 128:(ci + 1) * 128])
                ci_d = gp.tile([128, 128], I16, tag="ci_d")
                nc.sync.dma_start(ci_d[:], cls_d16[:, ci * 128:(ci + 1) * 128])
                gs = gp.tile([128, 1, 2048], BF16, tag="cgs")
                nc.gpsimd.dma_gather(
                    gs[:], xn3bf[:], ci_s[:], 2048, 2048, 128, transpose=True,
                    single_packet=False)
                gd = gp.tile([128, 1, 2048], BF16, tag="cgd")
                nc.gpsimd.dma_gather(
                    gd[:], xn3bf[:], ci_d[:], 2048, 2048, 128, transpose=True,
                    single_packet=False)
                eft = gp.tile([32, 4, 512], BF16, tag="ceft")
                nc.sync.dma_start(
                    eft[:], ef_store[ci].rearrange("g f e -> f g e")[:])
                for j in range(4):
                    sl = slice(j * 512, (j + 1) * 512)
                    pc1 = ps_a.tile([64, 512], F32, space="PSUM", tag="mm")
                    nc.tensor.matmul(pc1[:], lhsT=t_ec1a[:], rhs=gs[0:64, 0, sl],
                                     start=True, stop=False)
                    nc.tensor.matmul(pc1[:], lhsT=t_ec1b[:], rhs=gd[0:64, 0, sl],
                                     start=False, stop=False)
                    nc.tensor.matmul(pc1[:], lhsT=t_ec1c[:],
                                     rhs=eft[:, j, :],
                                     start=False, stop=True)
                    h1 = wp.tile([64, 512], BF16, tag="ch1")
                    nc.scalar.activation(h1[:], pc1[:], AF.Relu, bias=t_ec1bp[:])
                    pc2 = ps_a.tile([64, 512], F32, space="PSUM", tag="mm")
                    nc.tensor.matmul(pc2[:], lhsT=t_ec2[:], rhs=h1[:],
                                     start=True, stop=True)
                    h2 = wp.tile([64, 512], BF16, tag="ch2")
                    nc.scalar.activation(h2[:], pc2[:], AF.Relu, bias=t_ec2bp[:])
                    pc3 = ps_a.tile([1, 512], F32, space="PSUM", tag="mm")
                    nc.tensor.matmul(pc3[:], lhsT=t_ec3[:], rhs=h2[:],
                                     start=True, stop=True)
                    epv = wp.tile([1, 512], F32, tag="rowout")
                    nc.vector.tensor_scalar(epv[:], pc3[:], t_ec3bp[:], None,
                                            op0=OP.add)
                    nc.sync.dma_start(
                        ep_out[:, ci * 2048 + j * 512:ci * 2048 + (j + 1) * 512],
                        epv[:])

            # ---------- graph pooling (GSH graphs per core)
            gz = pp.tile([128, d.NBLK, HID], F32)
            nc.gpsimd.dma_gather(gz[:], xn3f[:], t_slotz16[:], d.NSLOT,
                                 d.NSLOT, HID, single_packet=False)
            gn = pp.tile([128, d.NBLK, HID], F32)
            nc.gpsimd.dma_gather(gn[:], xn3f[:], t_slotn16[:], d.NSLOT,
                                 d.NSLOT, HID, single_packet=False)
            psg = ps_a.tile([GSH, HID], F32, space="PSUM", tag="mm")
            for b in range(d.NBLK):
                nc.tensor.matmul(psg[:], lhsT=t_gsel[:, b, :], rhs=gz[:, b, :],
                                 start=(b == 0), stop=(b == d.NBLK - 1))
            gf = wp.tile([GSH, 2 * HID], F32, tag="gf")
            nc.scalar.activation(gf[:, 0:HID], psg[:], AF.Copy, scale=t_invc[:])
            bpg = d.SLOTG // 128  # blocks per graph
            gm8 = wp.tile([64, GSH], F32, tag="gm8")
            for g in range(GSH):
                gmt = wp.tile([64, d.SLOTG], F32, tag="gmt")
                for b2 in range(bpg):
                    b = g * bpg + b2
                    ptr = ps_c.tile([128, 128], F32, space="PSUM", tag="tr")
                    nc.tensor.transpose(ptr[0:HID, :], gn[:, b, :], t_ident[:])
                    nc.scalar.activation(gmt[:, b2 * 128:(b2 + 1) * 128],
                                         ptr[0:HID, :], AF.Copy)
                nc.vector.tensor_reduce(
                    gm8[:, g:g + 1], gmt[:],
                    axis=mybir.AxisListType.X, op=OP.max)
            ptg = ps_c.tile([128, 128], F32, space="PSUM", tag="tr")
            nc.tensor.transpose(ptg[0:GSH, 0:64], gm8[:, 0:GSH],
                                t_ident[0:64, 0:64])
            nc.scalar.activation(gf[:, HID:2 * HID], ptg[0:GSH, 0:64], AF.Copy)
            nc.sync.dma_start(gf_out[:], gf[:])

    nc.compile()
    return nc


# ---------------------------------------------------------------- host prep

def host_prep(x, edge_attr, edge_index, batch, params, d: Dims):
    """Build per-core input maps. Pure index/layout/weight-fusion work."""
    N, E, G = d.N, d.E, d.G
    NSH, NT, T_FIX, B_FIX = d.NSH, d.NT, d.T_FIX, d.B_FIX
    p = params

    src0 = np.asarray(edge_index[0], np.int64)
    dst0 = np.asarray(edge_index[1], np.int64)
    batch = np.asarray(batch, np.int64)
    loops = np.arange(N, dtype=np.int64)
    src_all = np.concatenate([src0, loops])
    dst_all = np.concatenate([dst0, loops])

    # ---- shared (weight) arrays
    def f32(a):
        return np.ascontiguousarray(np.asarray(a, np.float32))

    def bc(v, rows=128):
        v = f32(v).reshape(1, -1)
        return np.ascontiguousarray(np.repeat(v, rows, 0))

    shared = {}
    shared["ident"] = np.eye(128, dtype=np.float32)
    shared["ne_w"] = f32(p["ne_w"])
    shared["ne_b_bc"] = bc(p["ne_b"])
    shared["ne_g_bc"] = bc(p["ne_g"])
    shared["ne_be_bc"] = bc(p["ne_beta"])
    shared["ee_w"] = f32(p["ee_w"])
    shared["ee_b4"] = np.tile(f32(p["ee_b"]), 4).reshape(128, 1)
    bd = np.zeros((128, 4), np.float32)
    for j in range(4):
        bd[32 * j:32 * (j + 1), j] = 1.0
    shared["bd_sum"] = bd
    shared["bd_bc"] = np.ascontiguousarray(bd.T)

    for i, L in enumerate(d.layers):
        W = f32(p[f"g{i}_w"])                     # [fin, nh*hc]
        a_s = f32(p[f"g{i}_as"])                  # [nh, hc]
        a_d = f32(p[f"g{i}_ad"])
        fin, nh, hc = L["fin"], L["nh"], L["hc"]
        Wr = W.reshape(fin, nh, hc)
        Was = np.einsum("fhc,hc->fh", Wr, a_s)
        Wad = np.einsum("fhc,hc->fh", Wr, a_d)
        Waug = np.concatenate([W, Was, Wad], 1)   # [fin, nout]
        pk = np.zeros((128, L["kc"], L["nout"]), np.float32)
        for k in range(L["kc"]):
            kw = min(128, fin - k * 128)
            pk[0:kw, k, :] = Waug[k * 128:k * 128 + kw, :]
        shared[f"W{i}"] = pk
        shared[f"b{i}_bc"] = bc(p[f"g{i}_b"])

    # edge classifier: fold ef layernorm affine (g, beta) into ec1
    ec1 = f32(p["ec1_w"])                         # [160, 64]
    ec1_b = f32(p["ec1_b"]).copy()
    g_ef = f32(p["ee_g"])
    be_ef = f32(p["ee_beta"])
    ec1_ef = ec1[128:160, :]
    ec1_b = ec1_b + be_ef @ ec1_ef
    ec1_ef_folded = g_ef[:, None] * ec1_ef
    import ml_dtypes
    shared["ec1a"] = ec1[0:64, :].astype(ml_dtypes.bfloat16)
    shared["ec1b"] = ec1[64:128, :].astype(ml_dtypes.bfloat16)
    shared["ec1c"] = ec1_ef_folded.astype(ml_dtypes.bfloat16)
    shared["ec1_bp"] = ec1_b.reshape(64, 1)
    shared["ec2"] = f32(p["ec2_w"]).astype(ml_dtypes.bfloat16)
    shared["ec2_bp"] = f32(p["ec2_b"]).reshape(64, 1)
    shared["ec3"] = f32(p["ec3_w"]).astype(ml_dtypes.bfloat16)
    shared["ec3_bp"] = f32(p["ec3_b"]).reshape(1, 1)
    shared["ns1"] = f32(p["ns1_w"])
    shared["ns1_bp"] = f32(p["ns1_b"]).reshape(64, 1)
    shared["ns2"] = f32(p["ns2_w"])
    shared["ns2_bp"] = f32(p["ns2_b"]).reshape(1, 1)

    # ---- per-core arrays
    in_maps = []
    order = np.argsort(dst_all, kind="stable")
    dst_sorted = dst_all[order]
    src_sorted = src_all[order]
    tile_of = dst_sorted // 128                  # global dst tile id

    # pooling slot lists (graph-contiguous since batch is sorted)
    counts = np.bincount(batch, minlength=G)
    starts = np.concatenate([[0], np.cumsum(counts)[:-1]])

    for c in range(NCORES):
        m = {}
        nlo = c * NSH
        # x / edge_attr slices (transposed)
        xs = np.zeros((d.NF, NSH), np.float32)
        ns = min(NSH, N - nlo) if nlo < N else 0
        if ns > 0:
            xs[:, 0:ns] = np.asarray(x, np.float32)[nlo:nlo + ns, :].T
        m["xT"] = xs
        elo = c * d.ESH
        ea = np.zeros((d.EF, d.ESHP), np.float32)
        ea[:, 0:d.ESH] = np.asarray(
            edge_attr, np.float32)[elo:elo + d.ESH, :].T
        m["eaT"] = ea

        # GAT edges for this core, grouped by dst tile, padded to T_FIX
        srcl = np.full(NT * T_FIX, d.ZROW, np.int64)
        dstl_loc = np.full(NT * T_FIX, d.ZAD, np.int64)
        dstl_f = np.full(NT * T_FIX, -1.0, np.float32)
        for t in range(NT):
            gt = c * NT + t
            sel = np.nonzero(tile_of == gt)[0]
            cnt = sel.size
            assert cnt <= T_FIX, f"tile edge count {cnt} > T_FIX {T_FIX}"
            o = t * T_FIX
            srcl[o:o + cnt] = src_sorted[sel]
            dstl_loc[o:o + cnt] = dst_sorted[sel] - nlo
            dstl_f[o:o + cnt] = (dst_sorted[sel] - gt * 128).astype(np.float32)
        m["src16"] = _idx16(srcl, NT * T_FIX)
        m["dst16"] = _idx16(dstl_loc, NT * T_FIX)
        # dstl layout: edge k of tile t at [k%128, t*B_FIX + k//128]
        dl = dstl_f.reshape(NT, d.B_FIX, 128)
        m["dstl"] = np.ascontiguousarray(
            dl.transpose(2, 0, 1).reshape(128, NT * d.B_FIX))

        # classifier edges
        cls_s = np.full(d.ESHP, d.ZROW, np.int64)
        cls_dd = np.full(d.ESHP, d.ZROW, np.int64)
        cls_s[0:d.ESH] = src0[elo:elo + d.ESH]
        cls_dd[0:d.ESH] = dst0[elo:elo + d.ESH]
        m["cls_s16"] = _idx16(cls_s, d.ESHP)
        m["cls_d16"] = _idx16(cls_dd, d.ESHP)

        # pooling slots for graphs [c*GSH, (c+1)*GSH)
        slz = np.full(d.NSLOT, d.ZROW, np.int64)
        sln = np.full(d.NSLOT, d.XNEG, np.int64)
        gsel = np.zeros((128, d.NBLK, d.GSH), np.float32)
        inv = np.zeros((d.GSH, 1), np.float32)
        for gi in range(d.GSH):
            g = c * d.GSH + gi
            cnt = int(counts[g])
            cnt_c = min(cnt, d.SLOTG)
            o = gi * d.SLOTG
            ids = np.arange(starts[g], starts[g] + cnt_c)
            slz[o:o + cnt_c] = ids
            sln[o:o + cnt_c] = ids
            inv[gi, 0] = 1.0 / max(cnt, 1)
            for s in range(d.SLOTG):
                k = o + s
                gsel[k % 128, k // 128, gi] = 1.0 if s < cnt_c else 0.0
        m["slotz16"] = _idx16(slz, d.NSLOT)
        m["slotn16"] = _idx16(sln, d.NSLOT)
        m["gsel"] = gsel
        m["invcnt"] = inv

        m.update(shared)
        in_maps.append(m)
    return in_maps


# ---------------------------------------------------------------- entry point

_CACHE = {}


def _get_program(d: Dims):
    key = d.sig()
    if key not in _CACHE:
        _CACHE[key] = build_program(d)
    return _CACHE[key]


def make_dims(x, edge_attr, edge_index, batch):
    N, NF = x.shape
    E, EF = edge_attr.shape
    batch = np.asarray(batch, np.int64)
    G = 64
    HID, H = 64, 4
    dst_all = np.concatenate(
        [np.asarray(edge_index[1], np.int64), np.arange(N, dtype=np.int64)])
    tile_cnt = np.bincount(dst_all // 128, minlength=_ceil(N, 128))
    T_FIX = max(int(tile_cnt.max()), 128)
    T_FIX = _ceil(T_FIX, 128) * 128
    counts = np.bincount(batch, minlength=G)
    SLOTG = max(_ceil(int(counts.max()), 128) * 128, 128)
    return Dims(N, E, G, NF, EF, HID, H, T_FIX, SLOTG)


def kernel(x, edge_attr, edge_index, batch, params):
    from concourse.bass_utils import run_bass_kernel_spmd

    x = np.asarray(x)
    edge_attr = np.asarray(edge_attr)
    edge_index = np.asarray(edge_index)
    batch_np = np.asarray(batch)
    d = make_dims(x, edge_attr, edge_index, batch_np)
    nc = _get_program(d)
    in_maps = host_prep(x, edge_attr, edge_index, batch_np, params, d)
    res = run_bass_kernel_spmd(nc, in_maps, core_ids=list(range(NCORES)))
    r = res.results

    N, E, G = d.N, d.E, d.G
    xn = np.concatenate([r[c]["xn_out"] for c in range(NCORES)], 0)[:N]
    node_pred = np.concatenate(
        [r[c]["np_out"][0] for c in range(NCORES)], 0)[:N]
    edge_pred = np.concatenate(
        [r[c]["ep_out"][0][:d.ESH] for c in range(NCORES)], 0)[:E]
    graph_feat = np.concatenate([r[c]["gf_out"] for c in range(NCORES)], 0)
    return (edge_pred.astype(np.float32), node_pred.astype(np.float32),
            graph_feat.astype(np.float32), xn.astype(np.float32))
